# revision 47
# baseline (speedup 1.0000x reference)
"""Multi-head self-attention (RoPE, causal) Bass kernel for 8 TRN2 NeuronCores.

Problem: x (2, 2048, 1024) f32, wqkv (3072, 1024), wo (1024, 1024).
  qkv = x @ wqkv.T ; RoPE(q, k) ; causal softmax attention (16 heads, hd=64);
  out = y @ wo.T.

Sharding: batch (2-way) x head-group (4-way) tensor parallel = 8 cores.
Each core computes a full (2048, 1024) partial output for its batch from its
4 heads; host sums the 4 partials per batch (bf16 partials, f32 host sum).

Structure (measured 155-156us vs the 204us fp32r baseline):
  - softmax exp on the scalar engine (~84us serial) is the bottleneck; the
    kernel is one flattened (wave, tile) pipeline that keeps the exp stream
    dense: scores+exp ("front") run LA=2 tiles ahead of P@V ("back"), and
    qkv/wo micro-groups (~1us of PE each) fill the PE at deadline-chosen
    front positions, including across wave/chunk boundaries.
  - startup: WQKV columns reordered [q01|k01|q23|k23|v] and loaded in an
    A (q01/k01) + B split; xt per 512-col chunk across three DMA queues
    (each queue sustains only ~90GB/s); the first q/k groups' contraction
    loops are interleaved and ordered by DMA arrival.
  - fp16 everywhere in the rope/scores path (fp32r runs the PE moving pass
    at half clock; the paired 64-deep head matmuls share the PE as
    half-array tiles); bf16 -1e9 mask matmuls for the causal diagonal;
    fp16 P and V with an ones-column accumulating the softmax denominator.
  - first PV matmul of each wave uses start=True (zeroes the psum bank the
    y_ps tile owns - validated on HW for this exact emission schedule);
    y_all -> yt via SBUF->SBUF transpose DMA (PE transposes fused into the
    last wave's normalize, with ob copies on the then-idle scalar engine);
    output partials in bf16, summed in f32 on the host.
"""
import sys

sys.path.insert(0, "/opt/trn_rl_repo")

import numpy as np

import concourse.bass as bass
import concourse.mybir as mybir
import concourse.tile as tile
from concourse import bacc, bass_utils
from concourse.masks import make_identity

B, L, D = 2, 2048, 1024
NH, HD = 16, 64
NCORES = 8
HPC = 4            # heads per core
LQB = 512          # Lq block per S^T unit
NLQ = L // LQB     # 4
NLT = L // 128     # 16
KT = D // 128      # 8 contraction tiles for projections

F32 = mybir.dt.float32
F16 = mybir.dt.float16
BF16 = mybir.dt.bfloat16

# WQKV column offset per rope group m (m=0,1: q pairs h01,h23; 2,3: k pairs)
MCOL = {0: 0, 2: 128, 1: 256, 3: 384}

_cache = {}


def build_nc(debug=False):
    nc = bacc.Bacc("TRN2", target_bir_lowering=False, debug=False)

    XT = nc.dram_tensor("XT", [D, L], F16, kind="ExternalInput")
    WQKV = nc.dram_tensor("WQKV", [D, 772], F16, kind="ExternalInput")
    WOT = nc.dram_tensor("WOT", [HPC * HD, D], F16, kind="ExternalInput")
    PERM = nc.dram_tensor("PERM", [128, 128], F16, kind="ExternalInput")
    CS = nc.dram_tensor("CS", [128, 2 * L], F16, kind="ExternalInput")
    MASKS = nc.dram_tensor("MASKS", [128, 256], BF16, kind="ExternalInput")
    OUT = nc.dram_tensor("OUT", [L, D], BF16, kind="ExternalOutput")

    with tile.TileContext(nc) as tc:
        with (
            tc.tile_pool(name="consts", bufs=1) as cpool,
            tc.tile_pool(name="weights", bufs=1) as wpool,
            tc.tile_pool(name="qkrot", bufs=1) as rotpool,
            tc.tile_pool(name="vsb", bufs=1) as vpool,
            tc.tile_pool(name="yall", bufs=1) as ypool,
            tc.tile_pool(name="ytr", bufs=1) as ytpool,
            tc.tile_pool(name="xt", bufs=1) as xpool,
            tc.tile_pool(name="tmps", bufs=4) as tpool,
            tc.tile_pool(name="raws", bufs=3) as rawpool,
            tc.tile_pool(name="pts", bufs=4) as ptpool,
            tc.tile_pool(name="outsb", bufs=3) as opool,
            tc.tile_pool(name="recs", bufs=4) as recpool,
            tc.tile_pool(name="psB", bufs=3, space="PSUM") as bigpool,
            tc.tile_pool(name="psY", bufs=1, space="PSUM") as psypool,
        ):
            # ---- input DMAs, split across idle sequencers ----------------
            wqkv_sb = [None] * KT
            xts = [[None] * KT for _ in range(NLQ)]
            cs_sb = [None] * NLQ

            perm_sb = cpool.tile([128, 128], F16, tag="perm")
            masks_sb = cpool.tile([128, 256], BF16, tag="masks")
            idn = cpool.tile([128, 128], F16, tag="idn")

            def load_xt(eng, j, k):
                t = xpool.tile([128, LQB], F16, tag=f"xt{j}_{k}",
                               name=f"xt{j}_{k}")
                eng.dma_start(t[:], XT[k * 128:(k + 1) * 128,
                                       j * LQB:(j + 1) * LQB])
                xts[j][k] = t

            # spread the wave-(0,*)-critical set (wqkv A-halves + xt chunk
            # 0) across three DMA queues: each queue sustains only ~90GB/s
            for k in range(KT):
                wqkv_sb[k] = wpool.tile([128, 772], F16, tag=f"wqkv{k}",
                                        name=f"wqkv{k}")
                nc.sync.dma_start(wqkv_sb[k][:, 0:256],
                                  WQKV[k * 128:(k + 1) * 128, 0:256])
            for k in range(4):
                load_xt(nc.gpsimd, 0, k)
            for k in range(4, KT):
                load_xt(nc.scalar, 0, k)
            cs_sb[0] = cpool.tile([128, 1024], F16, tag="cs0", name="cs0")
            nc.sync.dma_start(cs_sb[0][:], CS[:, 0:1024])
            nc.scalar.dma_start(perm_sb[:], PERM[:, :])

            # B-halves (q23|k23|v) split across the SP and scalar queues;
            # xt chunk 1 on gpsimd
            for k in range(4):
                nc.sync.dma_start(wqkv_sb[k][:, 256:772],
                                  WQKV[k * 128:(k + 1) * 128, 256:772])
            for k in range(4, KT):
                nc.scalar.dma_start(wqkv_sb[k][:, 256:772],
                                    WQKV[k * 128:(k + 1) * 128, 256:772])
            for k in range(KT):
                load_xt(nc.gpsimd, 1, k)
            for j in range(1, NLQ):
                c = cpool.tile([128, 1024], F16, tag=f"cs{j}", name=f"cs{j}")
                nc.scalar.dma_start(c[:], CS[:, j * 1024:(j + 1) * 1024])
                cs_sb[j] = c
            nc.sync.dma_start(masks_sb[:], MASKS[:, :])
            wot_sb = []
            for c2 in range(2):
                w = wpool.tile([128, D], F16, tag=f"wo{c2}", name=f"wo{c2}")
                nc.sync.dma_start(w[:], WOT[c2 * 128:(c2 + 1) * 128, :])
                wot_sb.append(w)
            for j in range(2, NLQ):
                for k in range(KT):
                    load_xt(nc.sync, j, k)
            make_identity(nc, idn[:])

            # persistent activation storage
            qk_rot = [rotpool.tile([128, L], F16, tag=f"rot{m}",
                                   name=f"rot{m}")
                      for m in range(4)]
            v_sb = [vpool.tile([128, 260], F16, tag=f"v{t}", name=f"v{t}")
                    for t in range(NLT)]
            y_all = [ypool.tile([128, HPC * HD], F16, tag=f"y{i}", name=f"y{i}")
                     for i in range(NLT)]
            yt_sb = [ytpool.tile([128, L], F16, tag=f"yt{c2}", name=f"yt{c2}")
                     for c2 in range(2)]

            def ps_copy(j, out, in_):
                # chunk-0 copies ride the then-idle scalar engine
                if j == 0:
                    nc.scalar.copy(out, in_)
                else:
                    nc.vector.tensor_copy(out, in_)

            def qkv_m_parts(j, m):
                """Split a q/k projection+rope group into two ~1us PE
                micro-fillers so interleaved scores tiles aren't delayed."""
                co = MCOL[m]
                st = {}

                def part_a():
                    big = bigpool.tile([128, 1024], F32, tag="big",
                                       name="big")
                    st["big"] = big
                    for k in range(4):
                        nc.tensor.matmul(
                            big[:, 0:512], wqkv_sb[k][:, co:co + 128],
                            xts[j][k][:], start=(k == 0), stop=False,
                        )

                def part_b():
                    ps = st["big"][:, 0:512]
                    for k in range(4, KT):
                        nc.tensor.matmul(
                            ps, wqkv_sb[k][:, co:co + 128], xts[j][k][:],
                            start=False, stop=(k == KT - 1),
                        )
                    rope_finish(j, m, st["big"])

                return part_a, part_b

            def qkv_m_group(j, m):
                for fn in qkv_m_parts(j, m):
                    fn()

            def rope_finish(j, m, big):
                xs = slice(j * LQB, (j + 1) * LQB)
                ps = big[:, 0:512]
                psw = big[:, 512:1024]
                raw = rawpool.tile([128, LQB], F16, tag="raw")
                ps_copy(j, raw[:], ps)
                t1 = tpool.tile([128, LQB], F16, tag="t1")
                nc.vector.tensor_mul(t1[:], raw[:], cs_sb[j][:, 0:512])
                nc.tensor.matmul(psw, perm_sb[:], raw[:],
                                 start=True, stop=True)
                t2 = tpool.tile([128, LQB], F16, tag="t2")
                nc.vector.tensor_mul(t2[:], psw, cs_sb[j][:, 512:1024])
                nc.vector.tensor_add(qk_rot[m][:, xs], t1[:], t2[:])

            def qkv0_head(ms=(0, 2)):
                """Chunk-0 q01/k01 groups with the k-loop interleaved and
                ordered by DMA arrival (two xt queues land k0-3 and k4-7 in
                parallel) so the PE consumes tiles as they land."""
                bigs = {}
                for m in ms:
                    bigs[m] = bigpool.tile([128, 1024], F32, tag="big",
                                           name="big")
                order = [0, 4, 1, 5, 2, 6, 3, 7]
                for idx, kk in enumerate(order):
                    for m in ms:
                        nc.tensor.matmul(
                            bigs[m][:, 0:512],
                            wqkv_sb[kk][:, MCOL[m]:MCOL[m] + 128],
                            xts[0][kk][:],
                            start=(idx == 0), stop=(idx == len(order) - 1),
                            skip_group_check=True,
                        )
                for m in ms:
                    rope_finish(0, m, bigs[m])

            def v_group(j, i2):
                ti = j * 4 + i2
                bigv = bigpool.tile([128, 1024], F32, tag="big", name="bigv")
                psv = bigv[:, 0:260]
                for k in range(KT):
                    nc.tensor.matmul(
                        psv, xts[j][k][:, i2 * 128:(i2 + 1) * 128],
                        wqkv_sb[k][:, 512:772],
                        start=(k == 0), stop=(k == KT - 1),
                    )
                ps_copy(j, v_sb[ti][:], psv)
                nc.vector.memset(v_sb[ti][:, 64:260:65], 1.0)

            # ---- attention: front (scores+exp) / back (P@V) pipeline ----
            y_ps = {}       # wave -> [tile, tile]
            pt_of = {}      # (wave, t) -> pt tile

            def front(w, t):
                jq, hp = w
                ks = slice(t * 128, (t + 1) * 128)
                diag = t >= 4 * jq
                off = max(0, t * 128 - jq * LQB)
                sp = bigpool.tile([128, 1024], F32, tag="big", name="sp")
                for h in range(2):
                    hs = slice(64 * h, 64 * h + 64)
                    nc.tensor.matmul(
                        sp[:, 512 * h + off:512 * h + 512],
                        qk_rot[2 + hp][hs, ks],
                        qk_rot[hp][hs, jq * LQB + off:(jq + 1) * LQB],
                        start=True, stop=not diag,
                    )
                if diag:
                    for h in range(2):
                        nc.tensor.matmul(
                            sp[:, 512 * h + off:512 * h + off + 128],
                            masks_sb[:, 0:128],
                            masks_sb[:, 128:256],
                            start=False, stop=True,
                        )
                pt = ptpool.tile([128, 1024], F16, tag="pt")
                nc.scalar.activation(
                    pt[:, off:1024], sp[:, off:1024],
                    mybir.ActivationFunctionType.Exp
                )
                pt_of[(w, t)] = pt

            def back(w, t):
                jq, hp = w
                nt = 4 * jq + 4
                off = max(0, t * 128 - jq * LQB)
                r = off // 128
                if t == 0:
                    y_ps[w] = [psypool.tile([128, 260], F32, tag=f"yps{h}",
                                            name=f"yps{h}", bufs=1)
                               for h in range(2)]
                pt = pt_of.pop((w, t))
                for h in range(2):
                    H = 2 * hp + h
                    for js in range(r, 4):
                        nc.tensor.matmul(
                            y_ps[w][h][:, 65 * js:65 * js + 65],
                            pt[:, 512 * h + 128 * js:512 * h + 128 * js + 128],
                            v_sb[t][:, 65 * H:65 * H + 65],
                            # first matmul of the wave zeroes the whole
                            # psum bank (the tile owns it); validated on HW
                            # for this exact emission schedule
                            start=(t == 0 and js == 0), stop=(t == nt - 1),
                            skip_group_check=True,
                        )

            def wave_end(w):
                jq, hp = w
                fused_tail = (w == (NLQ - 1, 1))
                recs = []
                for h in range(2):
                    rec = recpool.tile([128, 4], F32, tag="rec")
                    nc.vector.reciprocal(rec[:], y_ps[w][h][:, 64:260:65])
                    recs.append(rec)
                for js in range(4):
                    i = 4 * jq + js
                    for h in range(2):
                        H = 2 * hp + h
                        nc.vector.tensor_scalar_mul(
                            y_all[i][:, HD * H:HD * H + HD],
                            y_ps[w][h][:, 65 * js:65 * js + 64],
                            recs[h][:, js:js + 1],
                        )
                    if fused_tail:
                        # last chunk: transpose + project + store this
                        # q-tile immediately (shortens the serial tail);
                        # ob copies on the now-idle scalar engine
                        transpose_one(i, pe=True)
                        wo_tile(i, ob_eng="scalar")
                del y_ps[w]

            def transpose_one(i, pe=False):
                if not pe:
                    for c2 in range(2):
                        nc.sync.dma_start(
                            yt_sb[c2][:, 128 * i:128 * i + 128],
                            y_all[i][:, 128 * c2:128 * c2 + 128],
                            transpose=True,
                        )
                else:
                    bigt = bigpool.tile([128, 1024], F32, tag="big",
                                        name="bigt")
                    tp16 = bigt[:].bitcast(F16)
                    for c2 in range(2):
                        nc.tensor.transpose(
                            tp16[:, 128 * c2:128 * c2 + 128],
                            y_all[i][:, 128 * c2:128 * c2 + 128],
                            idn[:],
                        )
                        nc.vector.tensor_copy(
                            yt_sb[c2][:, 128 * i:128 * i + 128],
                            tp16[:, 128 * c2:128 * c2 + 128],
                        )

            def transpose_tiles(jq):
                for i in range(4 * jq, 4 * jq + 4):
                    transpose_one(i)

            def wo_tile(i, ob_eng=None):
                po = bigpool.tile([128, 1024], F32, tag="big", name="po")
                for half in range(2):
                    for c2 in range(2):
                        nc.tensor.matmul(
                            po[:, 512 * half:512 * half + 512],
                            yt_sb[c2][:, 128 * i:128 * i + 128],
                            wot_sb[c2][:, 512 * half:512 * half + 512],
                            start=(c2 == 0), stop=(c2 == 1),
                        )
                ob = opool.tile([128, 1024], BF16, tag="ob")
                if ob_eng == "scalar":
                    nc.scalar.copy(ob[:], po[:])
                else:
                    nc.vector.tensor_copy(ob[:], po[:])
                nc.sync.dma_start(OUT[128 * i:128 * i + 128, :], ob[:])

            # ---- main schedule: flattened lookahead pipeline ------------
            LA = 2
            waves = [(jq, hp) for jq in range(NLQ) for hp in range(2)]
            nts = {w: 4 * w[0] + 4 for w in waves}
            seq = [(w, t) for w in waves for t in range(nts[w])]
            # window jq -> offset of its first front position
            woff = {}
            p = 0
            for w in waves:
                if w[1] == 0:
                    woff[w[0]] = p
                p += nts[w]

            def V(j, i2):
                return lambda: v_group(j, i2)

            def WO(i):
                return lambda: wo_tile(i)

            # explicit deadline-aware micro-filler positions
            # (window-relative; each item <= ~1us of PE time)
            def win_filler_table():
                MP = {(j, m): qkv_m_parts(j, m)
                      for j in range(NLQ) for m in range(4)
                      if (j, m) not in ((0, 0), (0, 2))}

                def MA(j, m):
                    return MP[(j, m)][0]

                def MB(j, m):
                    return MP[(j, m)][1]

                return {
                    0: [(0, V(0, 0)), (0, MA(0, 1)), (1, MB(0, 1)),
                        (2, MA(0, 3)), (2, V(0, 1)), (3, MB(0, 3)),
                        (4, V(0, 2)), (5, V(0, 3)),
                        (6, MA(1, 0)), (6, MB(1, 0)),
                        (7, MA(1, 2)), (7, MB(1, 2))],
                    1: [(0, MA(1, 1)), (1, MB(1, 1)), (2, V(1, 0)),
                        (3, MA(1, 3)), (4, MB(1, 3)), (5, V(1, 1)),
                        (6, V(1, 2)), (7, V(1, 3)),
                        (9, MA(2, 0)), (10, MB(2, 0)),
                        (12, MA(2, 2)), (13, MB(2, 2))],
                    2: [(0, MA(2, 1)), (1, MB(2, 1)), (2, V(2, 0)),
                        (3, MA(2, 3)), (4, MB(2, 3)), (5, V(2, 1)),
                        (7, V(2, 2)), (9, V(2, 3)),
                        (11, MA(3, 0)), (12, MB(3, 0)),
                        (14, MA(3, 2)), (15, MB(3, 2)),
                        (17, WO(0)), (19, WO(1)), (21, WO(2)), (23, WO(3))],
                    3: [(0, MA(3, 1)), (1, MB(3, 1)), (2, V(3, 0)),
                        (3, MA(3, 3)), (4, MB(3, 3)), (5, V(3, 1)),
                        (7, V(3, 2)), (9, V(3, 3)),
                        (12, WO(4)), (14, WO(5)), (16, WO(6)), (18, WO(7)),
                        (20, WO(8)), (22, WO(9)), (24, WO(10)), (26, WO(11))],
                }

            fpos = {}
            for jq, fl in win_filler_table().items():
                for rel, fn in fl:
                    fpos.setdefault(woff[jq] + rel, []).append(fn)

            def emit_back(q):
                wq, tq = seq[q]
                back(wq, tq)
                if tq == nts[wq] - 1:
                    wave_end(wq)
                    if wq[1] == 1 and wq[0] < NLQ - 1:
                        transpose_tiles(wq[0])

            qkv0_head()
            for p, (w, t) in enumerate(seq):
                for fn in fpos.get(p, ()):
                    fn()
                front(w, t)
                if p - LA >= 0:
                    emit_back(p - LA)
            for q in range(len(seq) - LA, len(seq)):
                emit_back(q)

    nc.finalize()
    return nc


def prep_inputs(x, wqkv, wo):
    """Build the 8 per-core input dicts from the full-problem inputs."""
    import ml_dtypes

    x = np.asarray(x, dtype=np.float32)
    wqkv = np.asarray(wqkv, dtype=np.float32)
    wo = np.asarray(wo, dtype=np.float32)

    # rope tables; CS: per chunk j a [cos_j (512) | sin_j (512)] block
    inv_freq = 1.0 / (10000.0 ** (np.arange(0, HD, 2, dtype=np.float32) / HD))
    t = np.arange(L, dtype=np.float32)
    freqs = np.outer(t, inv_freq)                  # (L, 32)
    cos32 = np.cos(freqs).T.astype(np.float32)     # (32, L)
    sin32 = np.sin(freqs).T.astype(np.float32)
    COS = np.tile(cos32, (4, 1))                                 # (128, L)
    SIN = np.concatenate([-sin32, sin32, -sin32, sin32], axis=0)
    CSfull = np.zeros((128, 2 * L), dtype=np.float32)
    for j in range(NLQ):
        CSfull[:, 1024 * j:1024 * j + 512] = COS[:, 512 * j:512 * j + 512]
        CSfull[:, 1024 * j + 512:1024 * j + 1024] = \
            SIN[:, 512 * j:512 * j + 512]

    # 32-block swap permutation (within each head's 64 rows)
    PERM = np.zeros((128, 128), dtype=np.float32)
    for blk in range(2):
        o = 64 * blk
        PERM[o:o + 32, o + 32:o + 64] = np.eye(32)
        PERM[o + 32:o + 64, o:o + 32] = np.eye(32)

    bf = ml_dtypes.bfloat16
    NEGI = (-1e9 * np.eye(128)).astype(np.float32)
    BIGM = (np.arange(128)[None, :] < np.arange(128)[:, None])
    MASKS = np.concatenate(
        [NEGI, BIGM.astype(np.float32)], axis=1).astype(bf)      # (128, 256)

    in_maps = []
    scale = np.float32(HD ** -0.5)
    for c in range(NCORES):
        b, g = divmod(c, 4)
        qrows = slice(256 * g, 256 * g + 256)
        krows = slice(1024 + 256 * g, 1024 + 256 * g + 256)
        vrows = slice(2048 + 256 * g, 2048 + 256 * g + 256)

        XT = np.ascontiguousarray(x[b].T)                        # (1024, 2048)
        wq = (wqkv[qrows, :] * scale).T                          # (1024, 256)
        wk = wqkv[krows, :].T
        vpart = wqkv[vrows, :].T                                 # (1024, 256)
        WV = np.zeros((D, 260), dtype=np.float32)
        for h in range(HPC):
            WV[:, 65 * h:65 * h + 64] = vpart[:, 64 * h:64 * h + 64]
        # columns: [q01 | k01 | q23 | k23 | v] so the first-needed (m=0,2)
        # groups are a contiguous 256-col A-half
        WQKV = np.ascontiguousarray(np.concatenate(
            [wq[:, 0:128], wk[:, 0:128], wq[:, 128:256], wk[:, 128:256], WV],
            axis=1))                                             # (1024, 772)
        WOT = np.ascontiguousarray(wo[:, 256 * g:256 * g + 256].T)

        in_maps.append({
            "XT": XT.astype(np.float16),
            "WQKV": WQKV.astype(np.float16),
            "WOT": WOT.astype(np.float16),
            "CS": CSfull.astype(np.float16),
            "PERM": PERM.astype(np.float16),
            "MASKS": MASKS,
        })
    return in_maps


def kernel(x, wqkv, wo):
    if "nc" not in _cache:
        _cache["nc"] = build_nc()
    nc = _cache["nc"]
    in_maps = prep_inputs(x, wqkv, wo)
    res = bass_utils.run_bass_kernel_spmd(nc, in_maps, list(range(NCORES)))
    outs = [np.asarray(res.results[c]["OUT"], dtype=np.float32)
            for c in range(NCORES)]
    out0 = outs[0] + outs[1] + outs[2] + outs[3]
    out1 = outs[4] + outs[5] + outs[6] + outs[7]
    return np.stack([out0, out1]).astype(np.float32)


# revision 48
# speedup vs baseline: 1.0107x; 1.0107x over previous
"""Multi-head self-attention (RoPE, causal) Bass kernel for 8 TRN2 NeuronCores.

Problem: x (2, 2048, 1024) f32, wqkv (3072, 1024), wo (1024, 1024).
  qkv = x @ wqkv.T ; RoPE(q, k) ; causal softmax attention (16 heads, hd=64);
  out = y @ wo.T.

Sharding: batch (2-way) x head-group (4-way) tensor parallel = 8 cores.
Each core computes a full (2048, 1024) partial output for its batch from its
4 heads; host sums the 4 partials per batch (bf16 partials, f32 host sum).

Structure (measured 155-156us vs the 204us fp32r baseline):
  - softmax exp on the scalar engine (~84us serial) is the bottleneck; the
    kernel is one flattened (wave, tile) pipeline that keeps the exp stream
    dense: scores+exp ("front") run LA=2 tiles ahead of P@V ("back"), and
    qkv/wo micro-groups (~1us of PE each) fill the PE at deadline-chosen
    front positions, including across wave/chunk boundaries.
  - startup: WQKV columns reordered [q01|k01|q23|k23|v] and loaded in an
    A (q01/k01) + B split; xt per 512-col chunk across three DMA queues
    (each queue sustains only ~90GB/s); the first q/k groups' contraction
    loops are interleaved and ordered by DMA arrival.
  - fp16 everywhere in the rope/scores path (fp32r runs the PE moving pass
    at half clock; the paired 64-deep head matmuls share the PE as
    half-array tiles); bf16 -1e9 mask matmuls for the causal diagonal;
    fp16 P and V with an ones-column accumulating the softmax denominator.
  - first PV matmul of each wave uses start=True (zeroes the psum bank the
    y_ps tile owns - validated on HW for this exact emission schedule);
    y_all -> yt via SBUF->SBUF transpose DMA (PE transposes fused into the
    last wave's normalize, with ob copies on the then-idle scalar engine);
    output partials in bf16, summed in f32 on the host.
"""
import sys

sys.path.insert(0, "/opt/trn_rl_repo")

import numpy as np

import concourse.bass as bass
import concourse.mybir as mybir
import concourse.tile as tile
from concourse import bacc, bass_utils
from concourse.masks import make_identity

B, L, D = 2, 2048, 1024
NH, HD = 16, 64
NCORES = 8
HPC = 4            # heads per core
LQB = 512          # Lq block per S^T unit
NLQ = L // LQB     # 4
NLT = L // 128     # 16
KT = D // 128      # 8 contraction tiles for projections

F32 = mybir.dt.float32
F16 = mybir.dt.float16
BF16 = mybir.dt.bfloat16

# WQKV column offset per rope group m (m=0,1: q pairs h01,h23; 2,3: k pairs)
MCOL = {0: 0, 2: 128, 1: 256, 3: 384}

_cache = {}


def build_nc(debug=False):
    nc = bacc.Bacc("TRN2", target_bir_lowering=False, debug=False)

    XT = nc.dram_tensor("XT", [D, L], F16, kind="ExternalInput")
    WQKV = nc.dram_tensor("WQKV", [D, 772], F16, kind="ExternalInput")
    WOT = nc.dram_tensor("WOT", [HPC * HD, D], F16, kind="ExternalInput")
    PERM = nc.dram_tensor("PERM", [128, 128], F16, kind="ExternalInput")
    CS = nc.dram_tensor("CS", [128, 2 * L], F16, kind="ExternalInput")
    MASKS = nc.dram_tensor("MASKS", [128, 256], BF16, kind="ExternalInput")
    OUT = nc.dram_tensor("OUT", [L, D], BF16, kind="ExternalOutput")

    with tile.TileContext(nc) as tc:
        with (
            tc.tile_pool(name="consts", bufs=1) as cpool,
            tc.tile_pool(name="weights", bufs=1) as wpool,
            tc.tile_pool(name="qkrot", bufs=1) as rotpool,
            tc.tile_pool(name="vsb", bufs=1) as vpool,
            tc.tile_pool(name="yall", bufs=1) as ypool,
            tc.tile_pool(name="ytr", bufs=1) as ytpool,
            tc.tile_pool(name="xt", bufs=1) as xpool,
            tc.tile_pool(name="tmps", bufs=4) as tpool,
            tc.tile_pool(name="raws", bufs=3) as rawpool,
            tc.tile_pool(name="pts", bufs=4) as ptpool,
            tc.tile_pool(name="outsb", bufs=3) as opool,
            tc.tile_pool(name="recs", bufs=4) as recpool,
            tc.tile_pool(name="psB", bufs=3, space="PSUM") as bigpool,
            tc.tile_pool(name="psY", bufs=1, space="PSUM") as psypool,
        ):
            # ---- input DMAs, split across idle sequencers ----------------
            wqkv_sb = [None] * KT
            xts = [[None] * KT for _ in range(NLQ)]
            cs_sb = [None] * NLQ

            perm_sb = cpool.tile([128, 128], F16, tag="perm")
            masks_sb = cpool.tile([128, 256], BF16, tag="masks")
            idn = cpool.tile([128, 128], F16, tag="idn")

            def load_xt(eng, j, k):
                t = xpool.tile([128, LQB], F16, tag=f"xt{j}_{k}",
                               name=f"xt{j}_{k}")
                eng.dma_start(t[:], XT[k * 128:(k + 1) * 128,
                                       j * LQB:(j + 1) * LQB])
                xts[j][k] = t

            # spread the wave-(0,*)-critical set (wqkv A-halves + xt chunk
            # 0) across three DMA queues: each queue sustains only ~90GB/s
            for k in range(KT):
                wqkv_sb[k] = wpool.tile([128, 772], F16, tag=f"wqkv{k}",
                                        name=f"wqkv{k}")
                nc.sync.dma_start(wqkv_sb[k][:, 0:256],
                                  WQKV[k * 128:(k + 1) * 128, 0:256])
            for k in range(4):
                load_xt(nc.gpsimd, 0, k)
            for k in range(4, KT):
                load_xt(nc.scalar, 0, k)
            cs_sb[0] = cpool.tile([128, 1024], F16, tag="cs0", name="cs0")
            nc.sync.dma_start(cs_sb[0][:], CS[:, 0:1024])
            nc.scalar.dma_start(perm_sb[:], PERM[:, :])

            # B-halves (q23|k23|v) split across the SP and scalar queues;
            # xt chunk 1 on gpsimd
            for k in range(4):
                nc.sync.dma_start(wqkv_sb[k][:, 256:772],
                                  WQKV[k * 128:(k + 1) * 128, 256:772])
            for k in range(4, KT):
                nc.scalar.dma_start(wqkv_sb[k][:, 256:772],
                                    WQKV[k * 128:(k + 1) * 128, 256:772])
            for k in range(KT):
                load_xt(nc.gpsimd, 1, k)
            for j in range(1, NLQ):
                c = cpool.tile([128, 1024], F16, tag=f"cs{j}", name=f"cs{j}")
                nc.scalar.dma_start(c[:], CS[:, j * 1024:(j + 1) * 1024])
                cs_sb[j] = c
            nc.sync.dma_start(masks_sb[:], MASKS[:, :])
            wot_sb = []
            for c2 in range(2):
                w = wpool.tile([128, D], F16, tag=f"wo{c2}", name=f"wo{c2}")
                nc.sync.dma_start(w[:], WOT[c2 * 128:(c2 + 1) * 128, :])
                wot_sb.append(w)
            for j in range(2, NLQ):
                for k in range(KT):
                    load_xt(nc.sync, j, k)
            make_identity(nc, idn[:])

            # persistent activation storage
            qk_rot = [rotpool.tile([128, L], F16, tag=f"rot{m}",
                                   name=f"rot{m}")
                      for m in range(4)]
            v_sb = [vpool.tile([128, 260], F16, tag=f"v{t}", name=f"v{t}")
                    for t in range(NLT)]
            y_all = [ypool.tile([128, HPC * HD], F16, tag=f"y{i}", name=f"y{i}")
                     for i in range(NLT)]
            yt_sb = [ytpool.tile([128, L], F16, tag=f"yt{c2}", name=f"yt{c2}")
                     for c2 in range(2)]

            def ps_copy(j, out, in_):
                # chunk-0 copies ride the then-idle scalar engine
                if j == 0:
                    nc.scalar.copy(out, in_)
                else:
                    nc.vector.tensor_copy(out, in_)

            def qkv_m_parts(j, m):
                """Split a q/k projection+rope group into two ~1us PE
                micro-fillers so interleaved scores tiles aren't delayed."""
                co = MCOL[m]
                st = {}

                def part_a():
                    big = bigpool.tile([128, 1024], F32, tag="big",
                                       name="big")
                    st["big"] = big
                    for k in range(4):
                        nc.tensor.matmul(
                            big[:, 0:512], wqkv_sb[k][:, co:co + 128],
                            xts[j][k][:], start=(k == 0), stop=False,
                        )

                def part_b():
                    ps = st["big"][:, 0:512]
                    for k in range(4, KT):
                        nc.tensor.matmul(
                            ps, wqkv_sb[k][:, co:co + 128], xts[j][k][:],
                            start=False, stop=(k == KT - 1),
                        )
                    rope_finish(j, m, st["big"])

                return part_a, part_b

            def qkv_m_group(j, m):
                for fn in qkv_m_parts(j, m):
                    fn()

            def rope_finish(j, m, big):
                xs = slice(j * LQB, (j + 1) * LQB)
                ps = big[:, 0:512]
                psw = big[:, 512:1024]
                raw = rawpool.tile([128, LQB], F16, tag="raw")
                ps_copy(j, raw[:], ps)
                t1 = tpool.tile([128, LQB], F16, tag="t1")
                nc.vector.tensor_mul(t1[:], raw[:], cs_sb[j][:, 0:512])
                nc.tensor.matmul(psw, perm_sb[:], raw[:],
                                 start=True, stop=True)
                t2 = tpool.tile([128, LQB], F16, tag="t2")
                nc.vector.tensor_mul(t2[:], psw, cs_sb[j][:, 512:1024])
                nc.vector.tensor_add(qk_rot[m][:, xs], t1[:], t2[:])

            def qkv0_head(ms=(0, 2)):
                """Chunk-0 q01/k01 groups with the k-loop interleaved and
                ordered by DMA arrival (two xt queues land k0-3 and k4-7 in
                parallel) so the PE consumes tiles as they land."""
                bigs = {}
                for m in ms:
                    bigs[m] = bigpool.tile([128, 1024], F32, tag="big",
                                           name="big")
                order = [0, 4, 1, 5, 2, 6, 3, 7]
                for idx, kk in enumerate(order):
                    for m in ms:
                        nc.tensor.matmul(
                            bigs[m][:, 0:512],
                            wqkv_sb[kk][:, MCOL[m]:MCOL[m] + 128],
                            xts[0][kk][:],
                            start=(idx == 0), stop=(idx == len(order) - 1),
                            skip_group_check=True,
                        )
                for m in ms:
                    rope_finish(0, m, bigs[m])

            def v_group(j, i2):
                ti = j * 4 + i2
                bigv = bigpool.tile([128, 1024], F32, tag="big", name="bigv")
                psv = bigv[:, 0:260]
                for k in range(KT):
                    nc.tensor.matmul(
                        psv, xts[j][k][:, i2 * 128:(i2 + 1) * 128],
                        wqkv_sb[k][:, 512:772],
                        start=(k == 0), stop=(k == KT - 1),
                    )
                ps_copy(j, v_sb[ti][:], psv)
                nc.vector.memset(v_sb[ti][:, 64:260:65], 1.0)

            # ---- attention: front (scores+exp) / back (P@V) pipeline ----
            y_ps = {}       # wave -> [tile, tile]
            pt_of = {}      # (wave, t) -> pt tile

            def front(w, t):
                jq, hp = w
                ks = slice(t * 128, (t + 1) * 128)
                diag = t >= 4 * jq
                off = max(0, t * 128 - jq * LQB)
                # early (PE-bound) windows zero the diag upper-triangle
                # post-exp on the idle gpsimd engine; late (exp-bound)
                # windows accumulate a -1e9 mask matmul on the PE, off the
                # exp critical path
                mask_mm = diag and jq >= 2
                sp = bigpool.tile([128, 1024], F32, tag="big", name="sp")
                for h in range(2):
                    hs = slice(64 * h, 64 * h + 64)
                    nc.tensor.matmul(
                        sp[:, 512 * h + off:512 * h + 512],
                        qk_rot[2 + hp][hs, ks],
                        qk_rot[hp][hs, jq * LQB + off:(jq + 1) * LQB],
                        start=True, stop=not mask_mm,
                    )
                if mask_mm:
                    for h in range(2):
                        nc.tensor.matmul(
                            sp[:, 512 * h + off:512 * h + off + 128],
                            masks_sb[:, 0:128],
                            masks_sb[:, 128:256],
                            start=False, stop=True,
                        )
                pt = ptpool.tile([128, 1024], F16, tag="pt")
                nc.scalar.activation(
                    pt[:, off:1024], sp[:, off:1024],
                    mybir.ActivationFunctionType.Exp
                )
                if diag and not mask_mm:
                    for h in range(2):
                        blk = pt[:, 512 * h + off:512 * h + off + 128]
                        nc.gpsimd.affine_select(
                            out=blk, in_=blk,
                            pattern=[[1, 128]],
                            compare_op=mybir.AluOpType.is_ge,
                            fill=0.0, base=0, channel_multiplier=-1,
                        )
                pt_of[(w, t)] = pt

            def back(w, t):
                jq, hp = w
                nt = 4 * jq + 4
                off = max(0, t * 128 - jq * LQB)
                r = off // 128
                if t == 0:
                    y_ps[w] = [psypool.tile([128, 260], F32, tag=f"yps{h}",
                                            name=f"yps{h}", bufs=1)
                               for h in range(2)]
                pt = pt_of.pop((w, t))
                for h in range(2):
                    H = 2 * hp + h
                    for js in range(r, 4):
                        nc.tensor.matmul(
                            y_ps[w][h][:, 65 * js:65 * js + 65],
                            pt[:, 512 * h + 128 * js:512 * h + 128 * js + 128],
                            v_sb[t][:, 65 * H:65 * H + 65],
                            # first matmul of the wave zeroes the whole
                            # psum bank (the tile owns it); validated on HW
                            # for this exact emission schedule
                            start=(t == 0 and js == 0), stop=(t == nt - 1),
                            skip_group_check=True,
                        )

            def wave_end(w):
                jq, hp = w
                fused_tail = (w == (NLQ - 1, 1))
                recs = []
                for h in range(2):
                    rec = recpool.tile([128, 4], F32, tag="rec")
                    nc.vector.reciprocal(rec[:], y_ps[w][h][:, 64:260:65])
                    recs.append(rec)
                for js in range(4):
                    i = 4 * jq + js
                    for h in range(2):
                        H = 2 * hp + h
                        nc.vector.tensor_scalar_mul(
                            y_all[i][:, HD * H:HD * H + HD],
                            y_ps[w][h][:, 65 * js:65 * js + 64],
                            recs[h][:, js:js + 1],
                        )
                    if fused_tail:
                        # last chunk: transpose + project + store this
                        # q-tile immediately (shortens the serial tail);
                        # ob copies on the now-idle scalar engine
                        transpose_one(i, pe=True)
                        wo_tile(i, ob_eng="scalar")
                del y_ps[w]

            def transpose_one(i, pe=False):
                if not pe:
                    for c2 in range(2):
                        nc.sync.dma_start(
                            yt_sb[c2][:, 128 * i:128 * i + 128],
                            y_all[i][:, 128 * c2:128 * c2 + 128],
                            transpose=True,
                        )
                else:
                    bigt = bigpool.tile([128, 1024], F32, tag="big",
                                        name="bigt")
                    tp16 = bigt[:].bitcast(F16)
                    for c2 in range(2):
                        nc.tensor.transpose(
                            tp16[:, 128 * c2:128 * c2 + 128],
                            y_all[i][:, 128 * c2:128 * c2 + 128],
                            idn[:],
                        )
                        nc.vector.tensor_copy(
                            yt_sb[c2][:, 128 * i:128 * i + 128],
                            tp16[:, 128 * c2:128 * c2 + 128],
                        )

            def transpose_tiles(jq):
                for i in range(4 * jq, 4 * jq + 4):
                    transpose_one(i)

            def wo_tile(i, ob_eng=None):
                po = bigpool.tile([128, 1024], F32, tag="big", name="po")
                for half in range(2):
                    for c2 in range(2):
                        nc.tensor.matmul(
                            po[:, 512 * half:512 * half + 512],
                            yt_sb[c2][:, 128 * i:128 * i + 128],
                            wot_sb[c2][:, 512 * half:512 * half + 512],
                            start=(c2 == 0), stop=(c2 == 1),
                        )
                ob = opool.tile([128, 1024], BF16, tag="ob")
                if ob_eng == "scalar":
                    nc.scalar.copy(ob[:], po[:])
                else:
                    nc.vector.tensor_copy(ob[:], po[:])
                nc.sync.dma_start(OUT[128 * i:128 * i + 128, :], ob[:])

            # ---- main schedule: flattened lookahead pipeline ------------
            LA = 2
            waves = [(jq, hp) for jq in range(NLQ) for hp in range(2)]
            nts = {w: 4 * w[0] + 4 for w in waves}
            seq = [(w, t) for w in waves for t in range(nts[w])]
            # window jq -> offset of its first front position
            woff = {}
            p = 0
            for w in waves:
                if w[1] == 0:
                    woff[w[0]] = p
                p += nts[w]

            def V(j, i2):
                return lambda: v_group(j, i2)

            def WO(i):
                return lambda: wo_tile(i)

            # explicit deadline-aware micro-filler positions
            # (window-relative; each item <= ~1us of PE time)
            def win_filler_table():
                MP = {(j, m): qkv_m_parts(j, m)
                      for j in range(NLQ) for m in range(4)
                      if (j, m) not in ((0, 0), (0, 2))}

                def MA(j, m):
                    return MP[(j, m)][0]

                def MB(j, m):
                    return MP[(j, m)][1]

                return {
                    0: [(0, V(0, 0)), (0, MA(0, 1)), (1, MB(0, 1)),
                        (2, MA(0, 3)), (2, V(0, 1)), (3, MB(0, 3)),
                        (4, V(0, 2)), (5, V(0, 3)),
                        (6, MA(1, 0)), (6, MB(1, 0)),
                        (7, MA(1, 2)), (7, MB(1, 2))],
                    1: [(0, MA(1, 1)), (1, MB(1, 1)), (2, V(1, 0)),
                        (3, MA(1, 3)), (4, MB(1, 3)), (5, V(1, 1)),
                        (6, V(1, 2)), (7, V(1, 3)),
                        (9, MA(2, 0)), (10, MB(2, 0)),
                        (12, MA(2, 2)), (13, MB(2, 2))],
                    2: [(0, MA(2, 1)), (1, MB(2, 1)), (2, V(2, 0)),
                        (3, MA(2, 3)), (4, MB(2, 3)), (5, V(2, 1)),
                        (7, V(2, 2)), (9, V(2, 3)),
                        (11, MA(3, 0)), (12, MB(3, 0)),
                        (14, MA(3, 2)), (15, MB(3, 2)),
                        (17, WO(0)), (19, WO(1)), (21, WO(2)), (23, WO(3))],
                    3: [(0, MA(3, 1)), (1, MB(3, 1)), (2, V(3, 0)),
                        (3, MA(3, 3)), (4, MB(3, 3)), (5, V(3, 1)),
                        (7, V(3, 2)), (9, V(3, 3)),
                        (12, WO(4)), (14, WO(5)), (16, WO(6)), (18, WO(7)),
                        (20, WO(8)), (22, WO(9)), (24, WO(10)), (26, WO(11))],
                }

            fpos = {}
            for jq, fl in win_filler_table().items():
                for rel, fn in fl:
                    fpos.setdefault(woff[jq] + rel, []).append(fn)

            def emit_back(q):
                wq, tq = seq[q]
                back(wq, tq)
                if tq == nts[wq] - 1:
                    wave_end(wq)
                    if wq[1] == 1 and wq[0] < NLQ - 1:
                        transpose_tiles(wq[0])

            qkv0_head()
            for p, (w, t) in enumerate(seq):
                for fn in fpos.get(p, ()):
                    fn()
                front(w, t)
                if p - LA >= 0:
                    emit_back(p - LA)
            for q in range(len(seq) - LA, len(seq)):
                emit_back(q)

    nc.finalize()
    return nc


def prep_inputs(x, wqkv, wo):
    """Build the 8 per-core input dicts from the full-problem inputs."""
    import ml_dtypes

    x = np.asarray(x, dtype=np.float32)
    wqkv = np.asarray(wqkv, dtype=np.float32)
    wo = np.asarray(wo, dtype=np.float32)

    # rope tables; CS: per chunk j a [cos_j (512) | sin_j (512)] block
    inv_freq = 1.0 / (10000.0 ** (np.arange(0, HD, 2, dtype=np.float32) / HD))
    t = np.arange(L, dtype=np.float32)
    freqs = np.outer(t, inv_freq)                  # (L, 32)
    cos32 = np.cos(freqs).T.astype(np.float32)     # (32, L)
    sin32 = np.sin(freqs).T.astype(np.float32)
    COS = np.tile(cos32, (4, 1))                                 # (128, L)
    SIN = np.concatenate([-sin32, sin32, -sin32, sin32], axis=0)
    CSfull = np.zeros((128, 2 * L), dtype=np.float32)
    for j in range(NLQ):
        CSfull[:, 1024 * j:1024 * j + 512] = COS[:, 512 * j:512 * j + 512]
        CSfull[:, 1024 * j + 512:1024 * j + 1024] = \
            SIN[:, 512 * j:512 * j + 512]

    # 32-block swap permutation (within each head's 64 rows)
    PERM = np.zeros((128, 128), dtype=np.float32)
    for blk in range(2):
        o = 64 * blk
        PERM[o:o + 32, o + 32:o + 64] = np.eye(32)
        PERM[o + 32:o + 64, o:o + 32] = np.eye(32)

    bf = ml_dtypes.bfloat16
    NEGI = (-1e9 * np.eye(128)).astype(np.float32)
    BIGM = (np.arange(128)[None, :] < np.arange(128)[:, None])
    MASKS = np.concatenate(
        [NEGI, BIGM.astype(np.float32)], axis=1).astype(bf)      # (128, 256)

    in_maps = []
    scale = np.float32(HD ** -0.5)
    for c in range(NCORES):
        b, g = divmod(c, 4)
        qrows = slice(256 * g, 256 * g + 256)
        krows = slice(1024 + 256 * g, 1024 + 256 * g + 256)
        vrows = slice(2048 + 256 * g, 2048 + 256 * g + 256)

        XT = np.ascontiguousarray(x[b].T)                        # (1024, 2048)
        wq = (wqkv[qrows, :] * scale).T                          # (1024, 256)
        wk = wqkv[krows, :].T
        vpart = wqkv[vrows, :].T                                 # (1024, 256)
        WV = np.zeros((D, 260), dtype=np.float32)
        for h in range(HPC):
            WV[:, 65 * h:65 * h + 64] = vpart[:, 64 * h:64 * h + 64]
        # columns: [q01 | k01 | q23 | k23 | v] so the first-needed (m=0,2)
        # groups are a contiguous 256-col A-half
        WQKV = np.ascontiguousarray(np.concatenate(
            [wq[:, 0:128], wk[:, 0:128], wq[:, 128:256], wk[:, 128:256], WV],
            axis=1))                                             # (1024, 772)
        WOT = np.ascontiguousarray(wo[:, 256 * g:256 * g + 256].T)

        in_maps.append({
            "XT": XT.astype(np.float16),
            "WQKV": WQKV.astype(np.float16),
            "WOT": WOT.astype(np.float16),
            "CS": CSfull.astype(np.float16),
            "PERM": PERM.astype(np.float16),
            "MASKS": MASKS,
        })
    return in_maps


def kernel(x, wqkv, wo):
    if "nc" not in _cache:
        _cache["nc"] = build_nc()
    nc = _cache["nc"]
    in_maps = prep_inputs(x, wqkv, wo)
    res = bass_utils.run_bass_kernel_spmd(nc, in_maps, list(range(NCORES)))
    outs = [np.asarray(res.results[c]["OUT"], dtype=np.float32)
            for c in range(NCORES)]
    out0 = outs[0] + outs[1] + outs[2] + outs[3]
    out1 = outs[4] + outs[5] + outs[6] + outs[7]
    return np.stack([out0, out1]).astype(np.float32)
